# revision 26
# baseline (speedup 1.0000x reference)
"""Trainium2 Bass kernel for a top-2 gated MoE layer (8 experts, H=1024, F=4096).

Strategy (expert parallelism across the 8 NeuronCores):
  - Host computes the top-2 routing (argsort of the fp32 gate logits) AND the
    combine weights comb = softmax(top2) * alpha, gathers each expert's tokens
    into a padded, transposed activation block xgT [H, C] (C = padded
    per-expert capacity).  All heavy math runs on device; the host only
    shards/gathers.
  - Each core runs one expert: LayerNorm + fc1 + gelu + fc2 + bias + gate
    scaling (matmuls in bf16 with fp32 PSUM accumulation, LN statistics in
    fp32).
  - Host scatter-adds the per-expert outputs back into the full [B,S,H]
    tensor.

Self-contained: shapes are hardcoded from the problem spec.
"""

import numpy as np
import ml_dtypes
from contextlib import ExitStack

TOP_K = 2
LN_EPS = 1e-5
B, S, H, E, F = 2, 2048, 1024, 8, 4096
T = B * S
P = 128
KH = H // P          # 8 H-tiles
FB = 1024            # F block size
NFB = F // FB        # 4 blocks
MF = FB // P         # 8 F-tiles per block

_BUILD_CACHE = {}


def _chunks(C):
    # Small first chunk so the LN -> fc1 pipeline fills quickly.
    out = []
    off = 0
    if C >= 768:
        out.append((0, 256))
        off = 256
    while C - off > 512:
        out.append((off, 512))
        off += 512
    if C - off:
        out.append((off, C - off))
    return out


def _build(C):
    """Build + compile the single-core Bass program (SPMD across 8 cores)."""
    if C in _BUILD_CACHE:
        return _BUILD_CACHE[C]

    import concourse.bass as bass  # noqa: F401
    import concourse.tile as tile
    import concourse.mybir as mybir
    from concourse import bacc, bass_isa

    bf = mybir.dt.bfloat16
    f32 = mybir.dt.float32
    AF = mybir.ActivationFunctionType
    OP = mybir.AluOpType

    nc = bacc.Bacc("TRN2", target_bir_lowering=False, debug=False, num_devices=8)

    d_xgT = nc.dram_tensor("xgT", [H, C], bf, kind="ExternalInput")
    d_w1 = nc.dram_tensor("w1", [H, F], bf, kind="ExternalInput")
    d_w2 = nc.dram_tensor("w2", [F, H], bf, kind="ExternalInput")
    d_b1r = nc.dram_tensor("b1r", [P, F // P], f32, kind="ExternalInput")
    d_pp = nc.dram_tensor("pp", [P, 3 * KH], f32, kind="ExternalInput")
    d_comb = nc.dram_tensor("comb", [1, C], bf, kind="ExternalInput")
    d_y = nc.dram_tensor("ytT", [H, C], bf, kind="ExternalOutput")

    chunks = _chunks(C)

    with tile.TileContext(nc) as tc, ExitStack() as ctx:
        const = ctx.enter_context(tc.tile_pool(name="const", bufs=1))
        gpool = ctx.enter_context(tc.tile_pool(name="gate", bufs=1))
        bpool = ctx.enter_context(tc.tile_pool(name="bcast", bufs=1))
        xpool = ctx.enter_context(tc.tile_pool(name="x", bufs=1))
        w1a = ctx.enter_context(tc.tile_pool(name="w1a", bufs=2))
        w2a = ctx.enter_context(tc.tile_pool(name="w2a", bufs=2))
        sqpool = ctx.enter_context(tc.tile_pool(name="sq", bufs=8))
        tpool = ctx.enter_context(tc.tile_pool(name="t1", bufs=3))
        hpool = ctx.enter_context(tc.tile_pool(name="hdn", bufs=KH))
        w1pool = ctx.enter_context(tc.tile_pool(name="w1", bufs=2))
        w2pool = ctx.enter_context(tc.tile_pool(name="w2", bufs=2))
        apool = ctx.enter_context(tc.tile_pool(name="acts", bufs=8))
        ypool = ctx.enter_context(tc.tile_pool(name="yacc", bufs=1))
        ps_small = ctx.enter_context(
            tc.tile_pool(name="ps_small", bufs=2, space="PSUM"))
        ps1 = ctx.enter_context(tc.tile_pool(name="ps1", bufs=3, space="PSUM"))
        ps2 = ctx.enter_context(tc.tile_pool(name="ps2", bufs=3, space="PSUM"))

        # ---- constants / small params ----
        ones_k = const.tile([P, 1], bf)
        nc.vector.memset(ones_k, 1.0)
        # all-ones [128,128] lhsT: column-reduces AND partition-broadcasts
        # in one matmul (out[m,c] = sum_k x[k,c] for every m)
        ones_mat = const.tile([P, P], bf)
        nc.vector.memset(ones_mat, 1.0)
        glib0 = const.tile([E, 1], f32)
        glib1 = const.tile([E, 1], f32)
        glibs = const.tile([E, 1], f32)
        glibr = const.tile([E, 1], f32)
        nc.vector.memset(glib0, 1.0)
        # pre-warm: gpsimd partition-op library, the Sqrt ACT table, and the
        # custom-DVE reciprocal uops — all during the initial x DMA wait, so
        # none of these one-time costs land on chunk 0's critical path
        nc.gpsimd.partition_broadcast(glib1[:], glib0[0:1, :], E)
        nc.scalar.activation(glibs[:], glib0[:], AF.Sqrt)
        nc.vector.reciprocal_approx_fast(out=glibr[:], in_=glib0[:])
        # PE warm-up: ~2us of junk matmuls trains the HAM clock gate toward
        # 2.4 GHz while the first x DMAs are still in flight.
        warm_rhs = const.tile([P, 512], bf)
        nc.vector.memset(warm_rhs, 0.0)
        ps_w = ps_small.tile([1, 512], f32, tag="pss", name="warm")
        for i in range(12):
            nc.tensor.matmul(ps_w[:], ones_k[:], warm_rhs[:],
                             start=True, stop=True)

        # ---- DMA issue order: x chunk0 -> w1 block0 (split) -> x rest ->
        # w2 block0 (split) -> small params.  fc1 of chunk 0 can start as
        # soon as the first w1 piece + chunk0 stats are in.
        xbig = xpool.tile([P, KH, C], bf, tag="xk", name="xbig")
        xk = [xbig[:, k, :] for k in range(KH)]
        d_xr = d_xgT.ap().rearrange("(k p) c -> p k c", p=P)
        nc.sync.dma_start(xbig[:, :, 0:chunks[0][1]],
                          d_xr[:, :, 0:chunks[0][1]])

        # block-0 weights split along the OUTPUT axis (f for w1, h for w2)
        # into SEPARATE tiles: DMA-completion semaphores are per-tile, so
        # the first fc1/fc2 psum groups start as soon as their piece lands.
        # interleave x chunks and w1 pieces in first-use order
        w1p0 = []
        for piece in range(2):
            t = w1a.tile([P, KH, 512], bf, tag="w1a", name=f"w1a_{piece}")
            w1p0.append(t)

        def load_w1_piece(piece):
            nc.sync.dma_start(
                w1p0[piece][:],
                d_w1.ap()[:, 512 * piece:512 * piece + 512].rearrange(
                    "(k p) f -> p k f", p=P))

        load_w1_piece(0)
        if len(chunks) > 1:
            off, w = chunks[1]
            nc.sync.dma_start(xbig[:, :, off:off + w], d_xr[:, :, off:off + w])
        load_w1_piece(1)
        for (off, w) in chunks[2:]:
            nc.sync.dma_start(xbig[:, :, off:off + w], d_xr[:, :, off:off + w])

        w2p0 = []
        for piece in range(2):
            t = w2a.tile([P, MF, 512], bf, tag="w2a", name=f"w2a_{piece}")
            nc.sync.dma_start(
                t[:],
                d_w2.ap()[0:FB, 512 * piece:512 * piece + 512].rearrange(
                    "(k p) h -> p k h", p=P))
            w2p0.append(t)

        pp_sb = const.tile([P, 3 * KH], f32)
        nc.sync.dma_start(pp_sb[:], d_pp.ap())
        lnw_sb = pp_sb[:, 0:KH]
        lnb_sb = pp_sb[:, KH:2 * KH]
        b2_sb = pp_sb[:, 2 * KH:3 * KH]
        b1_sb = const.tile([P, F // P], f32)
        nc.sync.dma_start(b1_sb[:], d_b1r.ap())
        comb_row = gpool.tile([1, C], bf)
        nc.sync.dma_start(comb_row[:], d_comb.ap())
        comb_b = bpool.tile([P, C], bf)
        nc.gpsimd.partition_broadcast(comb_b[:], comb_row[:], P)

        # ---- Phases A-C, pipelined along C-chunks so the PE can start the
        # fc1 matmuls of chunk 0 while later chunks are still in LN ----
        eps_t = gpool.tile([P, 1], f32)
        nc.vector.memset(eps_t, float(LN_EPS))
        # negated ln_w columns: hdn = ((mean_b - x) * -lnw) * inv + lnb
        nlnw_sb = gpool.tile([P, KH], f32)
        nc.scalar.activation(nlnw_sb[:], lnw_sb[:], AF.Identity,
                             bias=0.0, scale=-1.0)
        hdn = [hpool.tile([P, C], bf, tag="hdn", name=f"hdn{k}")
               for k in range(KH)]
        m_b = bpool.tile([P, C], bf)
        inv_b = bpool.tile([P, C], f32)

        ybig = ypool.tile([P, KH, C], bf, tag="yacc", name="ybig")
        y_acc = [ybig[:, h, :] for h in range(KH)]
        d_yr = d_y.ap().rearrange("(k p) c -> p k c", p=P)

        def load_w_block(fb):
            w1blk = w1pool.tile([P, KH, FB], bf, tag="w1", name=f"w1_{fb}")
            nc.sync.dma_start(
                w1blk[:],
                d_w1.ap()[:, fb * FB:(fb + 1) * FB].rearrange(
                    "(k p) f -> p k f", p=P))
            w2blk = w2pool.tile([P, MF, H], bf, tag="w2", name=f"w2_{fb}")
            nc.sync.dma_start(
                w2blk[:],
                d_w2.ap()[fb * FB:(fb + 1) * FB, :].rearrange(
                    "(k p) h -> p k h", p=P))
            return ([w1blk[:, k, :] for k in range(KH)],
                    [w2blk[:, k, :] for k in range(MF)])

        at0 = [apool.tile([P, C], bf, tag="acts", name=f"a_0_{m}")
               for m in range(MF)]

        def emit_prologue(ci):
            off, w = chunks[ci]
            sl = slice(off, off + w)
            # x^2 split between DVE and ACT so neither engine gates the chain
            sqs = []
            for k in range(KH):
                sq_c = sqpool.tile([P, w], bf, tag="sq", name=f"sq_{off}_{k}")
                if k < 4:
                    nc.vector.tensor_mul(sq_c[:], xk[k][:, sl], xk[k][:, sl])
                else:
                    nc.scalar.activation(sq_c[:], xk[k][:, sl], AF.Square)
                sqs.append(sq_c)
            # fold the 8 H-tiles on the DVE (bf16 partials), then a single
            # all-ones matmul per stat reduces across partitions AND
            # broadcasts the result to every partition
            px = tpool.tile([P, w], bf, tag="px", name=f"px_{off}")
            psq = tpool.tile([P, w], bf, tag="psq", name=f"psq_{off}")
            nc.vector.tensor_add(px[:], xk[0][:, sl], xk[1][:, sl])
            for k in range(2, KH):
                nc.vector.tensor_add(px[:], px[:], xk[k][:, sl])
            nc.vector.tensor_add(psq[:], sqs[0][:], sqs[1][:])
            for k in range(2, KH):
                nc.vector.tensor_add(psq[:], psq[:], sqs[k][:])
            ps_a = ps_small.tile([P, w], f32, tag="pss", name=f"ps_sl{off}")
            nc.tensor.matmul(ps_a[:], ones_mat[:], px[:], start=True, stop=True)
            nc.scalar.activation(m_b[:, sl], ps_a[:], AF.Identity,
                                 bias=0.0, scale=1.0 / H)
            ps_b = ps_small.tile([P, w], f32, tag="pss", name=f"ps_sq{off}")
            nc.tensor.matmul(ps_b[:], ones_mat[:], psq[:], start=True, stop=True)

            # var = sumsq/H - mean^2 ; inv = 1/sqrt(var + eps)
            v0 = tpool.tile([P, w], f32, tag="t1", name=f"v0_{off}")
            nc.vector.scalar_tensor_tensor(v0[:], m_b[:, sl], -1.0,
                                           m_b[:, sl], OP.mult, OP.mult)
            nc.vector.scalar_tensor_tensor(v0[:], ps_b[:], 1.0 / H,
                                           v0[:], OP.mult, OP.add)
            nc.scalar.activation(v0[:], v0[:], AF.Sqrt,
                                 bias=eps_t[:], scale=1.0)
            nc.vector.reciprocal_approx_fast(out=inv_b[:, sl], in_=v0[:])

            # apply LayerNorm -> hdn (bf16):
            #   hdn = ((m_b - x) * -lnw) * inv + lnb
            for k in range(KH):
                t1 = tpool.tile([P, w], bf, tag="t1b", name=f"t1_{off}_{k}")
                nc.vector.tensor_sub(t1[:], m_b[:, sl], xk[k][:, sl])
                nc.vector.scalar_tensor_tensor(t1[:], t1[:],
                                               nlnw_sb[:, k:k + 1],
                                               inv_b[:, sl], OP.mult, OP.mult)
                nc.scalar.activation(hdn[k][:, sl], t1[:], AF.Identity,
                                     bias=lnb_sb[:, k:k + 1], scale=1.0)

        def emit_fb0(ci):
            # F-block 0 fc1 -> gelu -> fc2 on this chunk (piece-split weights)
            off, w = chunks[ci]
            sl = slice(off, off + w)
            for m in range(MF):
                w1piece = w1p0[m // 4]
                msl = slice((m % 4) * P, (m % 4) * P + P)
                pst = ps1.tile([P, w], f32, tag="ps1", name=f"ps1_0_{m}_{ci}")
                for k in range(KH):
                    nc.tensor.matmul(pst[:], w1piece[:, k, msl],
                                     hdn[k][:, sl],
                                     start=(k == 0), stop=(k == KH - 1))
                nc.scalar.activation(at0[m][:, sl], pst[:],
                                     AF.Gelu_apprx_tanh,
                                     bias=b1_sb[:, m:m + 1])
            for h in range(KH):
                w2piece = w2p0[h // 4]
                hsl = slice((h % 4) * P, (h % 4) * P + P)
                pst = ps2.tile([P, w], f32, tag="ps2", name=f"ps2_0_{h}_{ci}")
                for k in range(MF):
                    nc.tensor.matmul(pst[:], w2piece[:, k, hsl],
                                     at0[k][:, sl],
                                     start=(k == 0), stop=(k == MF - 1))
                nc.scalar.activation(y_acc[h][:, sl], pst[:], AF.Identity,
                                     bias=0.0)

        # natural order: the x-chunk DMA gates each prologue anyway, and
        # emitting prologue(ci+1) ahead of fb0(ci) would stall the in-order
        # ACT queue (gelus of fb0(ci) behind chunk ci+1's sq/hdn ops).
        for ci in range(len(chunks)):
            emit_prologue(ci)
            emit_fb0(ci)

        # ---- Phase D: remaining F blocks.  Middle blocks iterate
        # weight-stationary (each lhsT feeds all chunks); the last block
        # iterates per-chunk so the finalize tail is short. ----
        for fb in range(1, NFB):
            w1t, w2t = load_w_block(fb)

            at = [apool.tile([P, C], bf, tag="acts", name=f"a_{fb}_{m}")
                  for m in range(MF)]
            if fb == NFB - 1:
                # per-range groups, widest first so the finalize tail is short
                groups = [[r] for r in sorted(chunks, key=lambda c: -c[1])]
            else:
                groups = [list(chunks)]

            for rg in groups:
                for m in range(MF):
                    psg = {r: ps1.tile([P, r[1]], f32, tag="ps1",
                                       name=f"ps1_{fb}_{m}_{r[0]}")
                           for r in rg}
                    for k in range(KH):
                        lhsT = w1t[k][:, m * P:(m + 1) * P]
                        for r in rg:
                            off, w = r
                            nc.tensor.matmul(psg[r][:], lhsT,
                                             hdn[k][:, off:off + w],
                                             start=(k == 0), stop=(k == KH - 1))
                    fcol = fb * MF + m
                    for r in rg:
                        off, w = r
                        nc.scalar.activation(at[m][:, off:off + w], psg[r][:],
                                             AF.Gelu_apprx_tanh,
                                             bias=b1_sb[:, fcol:fcol + 1])
                for h in range(KH):
                    psg = {r: ps2.tile([P, r[1]], f32, tag="ps2",
                                       name=f"ps2_{fb}_{h}_{r[0]}")
                           for r in rg}
                    for k in range(MF):
                        lhsT = w2t[k][:, h * P:(h + 1) * P]
                        for r in rg:
                            off, w = r
                            nc.tensor.matmul(psg[r][:], lhsT,
                                             at[k][:, off:off + w],
                                             start=(k == 0), stop=(k == MF - 1))
                    for r in rg:
                        off, w = r
                        if fb < NFB - 1:
                            nc.vector.tensor_add(y_acc[h][:, off:off + w],
                                                 y_acc[h][:, off:off + w],
                                                 psg[r][:])
                        else:
                            # fused finalize: y = (psum + b2) + y_acc, then
                            # scale by the gate weight and store this chunk
                            nc.vector.scalar_tensor_tensor(
                                y_acc[h][:, off:off + w], psg[r][:],
                                b2_sb[:, h:h + 1], y_acc[h][:, off:off + w],
                                OP.add, OP.add)
                            nc.vector.tensor_mul(y_acc[h][:, off:off + w],
                                                 y_acc[h][:, off:off + w],
                                                 comb_b[:, off:off + w])
                            nc.sync.dma_start(
                                d_yr[:, h:h + 1, off:off + w],
                                ybig[:, h:h + 1, off:off + w])

    nc.compile()
    _BUILD_CACHE[C] = nc
    return nc


def _prepare(x, Wg, alpha, ln_w, ln_b, fc1_w, fc1_b, fc2_w, fc2_b):
    """Host-side routing + per-core input construction."""
    bfnp = ml_dtypes.bfloat16
    xf = np.asarray(x, np.float32).reshape(T, H)
    Wg = np.asarray(Wg, np.float32)
    alpha = np.asarray(alpha, np.float32)
    ln_w = np.asarray(ln_w, np.float32)
    ln_b = np.asarray(ln_b, np.float32)
    fc1_w = np.asarray(fc1_w, np.float32)
    fc1_b = np.asarray(fc1_b, np.float32)
    fc2_w = np.asarray(fc2_w, np.float32)
    fc2_b = np.asarray(fc2_b, np.float32)

    logits = xf @ Wg
    order = np.argsort(-logits, axis=1, kind="stable")
    top2 = order[:, :TOP_K]
    tv = np.take_along_axis(logits, top2, 1)
    sm = np.exp(tv - tv.max(1, keepdims=True))
    sm /= sm.sum(1, keepdims=True)
    comb = np.zeros((T, E), np.float32)
    np.put_along_axis(comb, top2, sm, 1)
    comb *= alpha
    sel = np.zeros((T, E), dtype=bool)
    sel[np.arange(T)[:, None], top2] = True
    idx = [np.nonzero(sel[:, e])[0] for e in range(E)]

    maxc = max(len(i) for i in idx)
    C = max(512, 16 * ((maxc + 15) // 16))

    KHp = H // 128
    in_maps = []
    for e in range(E):
        n = len(idx[e])
        xg = np.zeros((C, H), np.float32)
        xg[:n] = xf[idx[e]]
        cv = np.zeros((1, C), bfnp)
        cv[0, :n] = comb[idx[e], e].astype(bfnp)
        pp = np.concatenate([
            ln_w[e].reshape(KHp, 128).T,
            ln_b[e].reshape(KHp, 128).T,
            fc2_b[e].reshape(KHp, 128).T,
        ], axis=1)
        in_maps.append({
            "xgT": np.ascontiguousarray(xg.T).astype(bfnp),
            "w1": fc1_w[e].astype(bfnp),
            "w2": fc2_w[e].astype(bfnp),
            "b1r": np.ascontiguousarray(fc1_b[e].reshape(F // 128, 128).T),
            "pp": np.ascontiguousarray(pp),
            "comb": cv,
        })
    return in_maps, idx, C


def _kernel_impl(inputs, trace=False, trace_cores=None):
    from concourse import bass_utils

    in_maps, idx, C = _prepare(**inputs)
    nc = _build(C)
    res = bass_utils.run_bass_kernel_spmd(
        nc, in_maps, core_ids=list(range(E)),
        trace=trace, trace_cores=trace_cores)

    out = np.zeros((T, H), np.float32)
    for e in range(E):
        yt = np.asarray(res.results[e]["ytT"], np.float32)  # [H, C]
        n = len(idx[e])
        out[idx[e]] += yt.T[:n]
    return out.reshape(B, S, H), res


def kernel(**inputs):
    out, _ = _kernel_impl(inputs)
    return out


# revision 29
# speedup vs baseline: 1.0047x; 1.0047x over previous
"""Trainium2 Bass kernel for a top-2 gated MoE layer (8 experts, H=1024, F=4096).

Strategy (expert parallelism across the 8 NeuronCores):
  - Host computes the top-2 routing AND the combine weights
    comb = softmax(top2) * alpha, gathers each expert's tokens into a padded,
    transposed activation block xgT [H, C] (C = padded per-expert capacity).
  - LayerNorm is folded into fc1 algebraically:
        hdn = (x - mu) * inv * lnw + lnb
        hdn @ W1 = inv * (x @ W1e - mu * s1) + (lnb @ W1)
    with W1e = lnw[:,None] * W1 and s1 = colsum(W1e) precomputed on host.
    So fc1 matmuls consume the raw x tiles (no LN on the matmul path); the
    per-token mean/inv statistics only enter a cheap post-matmul fixup on
    the PSUM output, right before the gelu.
  - Each core runs one expert (matmuls in bf16, fp32 PSUM accumulation).
  - Host scatter-adds the per-expert outputs back into the full [B,S,H].

Self-contained: shapes are hardcoded from the problem spec.
"""

import numpy as np
import ml_dtypes
from contextlib import ExitStack

TOP_K = 2
LN_EPS = 1e-5
B, S, H, E, F = 2, 2048, 1024, 8, 4096
T = B * S
P = 128
KH = H // P          # 8 H-tiles
FB = 1024            # F block size
NFB = F // FB        # 4 blocks
MF = FB // P         # 8 F-tiles per block

_BUILD_CACHE = {}


def _chunks(C):
    # First chunk sized so fb0(c0) PE work covers the weight DMA stream;
    # the remainder chunk lands last (small finalize tail).
    out = []
    off = 0
    if C >= 960:
        out.append((0, 448))
        off = 448
    while C - off > 512:
        out.append((off, 512))
        off += 512
    if C - off:
        out.append((off, C - off))
    return out


def _build(C):
    """Build + compile the single-core Bass program (SPMD across 8 cores)."""
    if C in _BUILD_CACHE:
        return _BUILD_CACHE[C]

    import concourse.bass as bass  # noqa: F401
    import concourse.tile as tile
    import concourse.mybir as mybir
    from concourse import bacc, bass_isa  # noqa: F401

    bf = mybir.dt.bfloat16
    f32 = mybir.dt.float32
    AF = mybir.ActivationFunctionType
    OP = mybir.AluOpType

    nc = bacc.Bacc("TRN2", target_bir_lowering=False, debug=False, num_devices=8)

    d_xgT = nc.dram_tensor("xgT", [H, C], bf, kind="ExternalInput")
    d_w1 = nc.dram_tensor("w1", [H, F], bf, kind="ExternalInput")
    d_w2 = nc.dram_tensor("w2", [F, H], bf, kind="ExternalInput")
    d_b1r = nc.dram_tensor("b1r", [P, F // P], f32, kind="ExternalInput")
    d_s1r = nc.dram_tensor("s1r", [P, F // P], f32, kind="ExternalInput")
    d_b2 = nc.dram_tensor("b2", [P, KH], f32, kind="ExternalInput")
    d_comb = nc.dram_tensor("comb", [1, C], bf, kind="ExternalInput")
    d_y = nc.dram_tensor("ytT", [H, C], bf, kind="ExternalOutput")

    chunks = _chunks(C)

    with tile.TileContext(nc) as tc, ExitStack() as ctx:
        const = ctx.enter_context(tc.tile_pool(name="const", bufs=1))
        gpool = ctx.enter_context(tc.tile_pool(name="gate", bufs=1))
        bpool = ctx.enter_context(tc.tile_pool(name="bcast", bufs=1))
        xpool = ctx.enter_context(tc.tile_pool(name="x", bufs=1))
        w1a = ctx.enter_context(tc.tile_pool(name="w1a", bufs=2))
        w2a = ctx.enter_context(tc.tile_pool(name="w2a", bufs=2))
        sqpool = ctx.enter_context(tc.tile_pool(name="sq", bufs=8))
        tpool = ctx.enter_context(tc.tile_pool(name="t1", bufs=3))
        fxpool = ctx.enter_context(tc.tile_pool(name="fx", bufs=4))
        w1pool = ctx.enter_context(tc.tile_pool(name="w1", bufs=2))
        w2pool = ctx.enter_context(tc.tile_pool(name="w2", bufs=2))
        apool = ctx.enter_context(tc.tile_pool(name="acts", bufs=8))
        ypool = ctx.enter_context(tc.tile_pool(name="yacc", bufs=1))
        ps_small = ctx.enter_context(
            tc.tile_pool(name="ps_small", bufs=2, space="PSUM"))
        ps1 = ctx.enter_context(tc.tile_pool(name="ps1", bufs=3, space="PSUM"))
        ps2 = ctx.enter_context(tc.tile_pool(name="ps2", bufs=3, space="PSUM"))

        # ---- constants / small params ----
        ones_k = const.tile([P, 1], bf)
        nc.vector.memset(ones_k, 1.0)
        # all-ones [128,128] lhsT: column-reduces AND partition-broadcasts
        # in one matmul (out[m,c] = sum_k x[k,c] for every m)
        ones_mat = const.tile([P, P], bf)
        nc.vector.memset(ones_mat, 1.0)
        glib0 = const.tile([E, 1], f32)
        glib1 = const.tile([E, 1], f32)
        glibs = const.tile([E, 1], f32)
        glibr = const.tile([E, 1], f32)
        nc.vector.memset(glib0, 1.0)
        # pre-warm: gpsimd partition-op library, the Sqrt ACT table, and the
        # custom-DVE reciprocal uops — all during the initial x DMA wait
        nc.gpsimd.partition_broadcast(glib1[:], glib0[0:1, :], E)
        nc.scalar.activation(glibs[:], glib0[:], AF.Sqrt)
        nc.vector.reciprocal_approx_fast(out=glibr[:], in_=glib0[:])
        # PE warm-up: junk matmuls train the HAM clock gate toward 2.4 GHz
        # while the first x/w DMAs are still in flight.
        warm_rhs = const.tile([P, 512], bf)
        nc.vector.memset(warm_rhs, 0.0)
        ps_w = ps_small.tile([1, 512], f32, tag="pss", name="warm")
        for i in range(12):
            nc.tensor.matmul(ps_w[:], ones_k[:], warm_rhs[:],
                             start=True, stop=True)

        # ---- DMA: x chunks and block-0 weight pieces in first-use order
        xbig = xpool.tile([P, KH, C], bf, tag="xk", name="xbig")
        xk = [xbig[:, k, :] for k in range(KH)]
        d_xr = d_xgT.ap().rearrange("(k p) c -> p k c", p=P)
        nc.sync.dma_start(xbig[:, :, 0:chunks[0][1]],
                          d_xr[:, :, 0:chunks[0][1]])

        w1p0 = []
        for piece in range(2):
            t = w1a.tile([P, KH, 512], bf, tag="w1a", name=f"w1a_{piece}")
            w1p0.append(t)

        def load_w1_piece(piece):
            nc.sync.dma_start(
                w1p0[piece][:],
                d_w1.ap()[:, 512 * piece:512 * piece + 512].rearrange(
                    "(k p) f -> p k f", p=P))

        w2p0 = []
        for piece in range(2):
            t = w2a.tile([P, MF, 512], bf, tag="w2a", name=f"w2a_{piece}")
            w2p0.append(t)

        def load_w2_piece(piece):
            nc.sync.dma_start(
                w2p0[piece][:],
                d_w2.ap()[0:FB, 512 * piece:512 * piece + 512].rearrange(
                    "(k p) h -> p k h", p=P))

        # strict byte-priority order on the sync ring, by first-use time
        load_w1_piece(0)
        if len(chunks) > 1:
            off = chunks[1][0]
            nc.sync.dma_start(xbig[:, :, off:C], d_xr[:, :, off:C])
        load_w2_piece(0)
        load_w1_piece(1)
        load_w2_piece(1)

        b2_sb = const.tile([P, KH], f32)
        nc.sync.dma_start(b2_sb[:], d_b2.ap())
        b1_sb = const.tile([P, F // P], f32)
        nc.sync.dma_start(b1_sb[:], d_b1r.ap())
        s1_sb = const.tile([P, F // P], f32)
        nc.sync.dma_start(s1_sb[:], d_s1r.ap())
        comb_row = gpool.tile([1, C], bf)
        nc.sync.dma_start(comb_row[:], d_comb.ap())
        comb_b = bpool.tile([P, C], bf)
        nc.gpsimd.partition_broadcast(comb_b[:], comb_row[:], P)

        eps_t = gpool.tile([P, 1], f32)
        nc.vector.memset(eps_t, float(LN_EPS))
        m_b = bpool.tile([P, C], bf)
        inv_b = bpool.tile([P, C], f32)

        ybig = ypool.tile([P, KH, C], bf, tag="yacc", name="ybig")
        y_acc = [ybig[:, h, :] for h in range(KH)]
        d_yr = d_y.ap().rearrange("(k p) c -> p k c", p=P)

        def load_w_block(fb):
            w1blk = w1pool.tile([P, KH, FB], bf, tag="w1", name=f"w1_{fb}")
            nc.sync.dma_start(
                w1blk[:],
                d_w1.ap()[:, fb * FB:(fb + 1) * FB].rearrange(
                    "(k p) f -> p k f", p=P))
            w2blk = w2pool.tile([P, MF, H], bf, tag="w2", name=f"w2_{fb}")
            nc.sync.dma_start(
                w2blk[:],
                d_w2.ap()[fb * FB:(fb + 1) * FB, :].rearrange(
                    "(k p) h -> p k h", p=P))
            return ([w1blk[:, k, :] for k in range(KH)],
                    [w2blk[:, k, :] for k in range(MF)])

        at0 = [apool.tile([P, C], bf, tag="acts", name=f"a_0_{m}")
               for m in range(MF)]

        def emit_prologue(ci):
            off, w = chunks[ci]
            sl = slice(off, off + w)
            # x^2 split between DVE and ACT so neither engine gates the chain
            sqs = []
            for k in range(KH):
                sq_c = sqpool.tile([P, w], bf, tag="sq", name=f"sq_{off}_{k}")
                if k < 4:
                    nc.vector.tensor_mul(sq_c[:], xk[k][:, sl], xk[k][:, sl])
                else:
                    nc.scalar.activation(sq_c[:], xk[k][:, sl], AF.Square)
                sqs.append(sq_c)
            # fold the 8 H-tiles on the DVE (bf16 partials), then a single
            # all-ones matmul per stat reduces across partitions AND
            # broadcasts the result to every partition
            px = tpool.tile([P, w], bf, tag="px", name=f"px_{off}")
            psq = tpool.tile([P, w], bf, tag="psq", name=f"psq_{off}")
            nc.vector.tensor_add(px[:], xk[0][:, sl], xk[1][:, sl])
            for k in range(2, KH):
                nc.vector.tensor_add(px[:], px[:], xk[k][:, sl])
            nc.vector.tensor_add(psq[:], sqs[0][:], sqs[1][:])
            for k in range(2, KH):
                nc.vector.tensor_add(psq[:], psq[:], sqs[k][:])
            ps_a = ps_small.tile([P, w], f32, tag="pss", name=f"ps_sl{off}")
            nc.tensor.matmul(ps_a[:], ones_mat[:], px[:], start=True, stop=True)
            nc.scalar.activation(m_b[:, sl], ps_a[:], AF.Identity,
                                 bias=0.0, scale=1.0 / H)
            ps_b = ps_small.tile([P, w], f32, tag="pss", name=f"ps_sq{off}")
            nc.tensor.matmul(ps_b[:], ones_mat[:], psq[:], start=True, stop=True)

            # var = sumsq/H - mean^2 ; inv = 1/sqrt(var + eps)
            v0 = tpool.tile([P, w], f32, tag="t1", name=f"v0_{off}")
            nc.vector.scalar_tensor_tensor(v0[:], m_b[:, sl], -1.0,
                                           m_b[:, sl], OP.mult, OP.mult)
            nc.vector.scalar_tensor_tensor(v0[:], ps_b[:], 1.0 / H,
                                           v0[:], OP.mult, OP.add)
            nc.scalar.activation(v0[:], v0[:], AF.Sqrt,
                                 bias=eps_t[:], scale=1.0)
            nc.vector.reciprocal_approx_fast(out=inv_b[:, sl], in_=v0[:])

        def fc1_fixup(psum, dst, off, w, fcol):
            # gelu( inv*(A - mu*s1) + b1' ):
            #   t = (m_b * s1_f) - A ; t = t * inv ; ACT gelu(-t + b1')
            sl = slice(off, off + w)
            tmp = fxpool.tile([P, w], f32, tag="fx", name=f"fx_{fcol}_{off}")
            nc.vector.scalar_tensor_tensor(tmp[:], m_b[:, sl],
                                           s1_sb[:, fcol:fcol + 1],
                                           psum[:], OP.mult, OP.subtract)
            nc.vector.tensor_mul(tmp[:], tmp[:], inv_b[:, sl])
            nc.scalar.activation(dst, tmp[:], AF.Gelu_apprx_tanh,
                                 bias=b1_sb[:, fcol:fcol + 1], scale=-1.0)

        def emit_fb0(ci):
            # F-block 0 fc1 -> fixup+gelu -> fc2 on this chunk
            off, w = chunks[ci]
            sl = slice(off, off + w)
            for m in range(MF):
                w1piece = w1p0[m // 4]
                msl = slice((m % 4) * P, (m % 4) * P + P)
                pst = ps1.tile([P, w], f32, tag="ps1", name=f"ps1_0_{m}_{ci}")
                for k in range(KH):
                    nc.tensor.matmul(pst[:], w1piece[:, k, msl],
                                     xk[k][:, sl],
                                     start=(k == 0), stop=(k == KH - 1))
                fc1_fixup(pst, at0[m][:, sl], off, w, m)
            for h in range(KH):
                w2piece = w2p0[h // 4]
                hsl = slice((h % 4) * P, (h % 4) * P + P)
                pst = ps2.tile([P, w], f32, tag="ps2", name=f"ps2_0_{h}_{ci}")
                for k in range(MF):
                    nc.tensor.matmul(pst[:], w2piece[:, k, hsl],
                                     at0[k][:, sl],
                                     start=(k == 0), stop=(k == MF - 1))
                nc.scalar.activation(y_acc[h][:, sl], pst[:], AF.Identity,
                                     bias=0.0)

        # natural order: the x-chunk DMA gates each prologue anyway, and the
        # stats only feed the post-matmul fixups, never the matmuls
        for ci in range(len(chunks)):
            emit_prologue(ci)
            emit_fb0(ci)

        # ---- Phase D: remaining F blocks.  Middle blocks iterate
        # weight-stationary (each lhsT feeds all chunks); the last block
        # iterates per-range so the finalize tail is short. ----
        for fb in range(1, NFB):
            w1t, w2t = load_w_block(fb)

            at = [apool.tile([P, C], bf, tag="acts", name=f"a_{fb}_{m}")
                  for m in range(MF)]
            if fb == NFB - 1:
                groups = [[r] for r in sorted(chunks, key=lambda c: -c[1])]
            else:
                groups = [list(chunks)]

            for rg in groups:
                for m in range(MF):
                    psg = {r: ps1.tile([P, r[1]], f32, tag="ps1",
                                       name=f"ps1_{fb}_{m}_{r[0]}")
                           for r in rg}
                    for k in range(KH):
                        lhsT = w1t[k][:, m * P:(m + 1) * P]
                        for r in rg:
                            off, w = r
                            nc.tensor.matmul(psg[r][:], lhsT,
                                             xk[k][:, off:off + w],
                                             start=(k == 0), stop=(k == KH - 1))
                    fcol = fb * MF + m
                    for r in rg:
                        off, w = r
                        fc1_fixup(psg[r], at[m][:, off:off + w], off, w, fcol)
                for h in range(KH):
                    psg = {r: ps2.tile([P, r[1]], f32, tag="ps2",
                                       name=f"ps2_{fb}_{h}_{r[0]}")
                           for r in rg}
                    for k in range(MF):
                        lhsT = w2t[k][:, h * P:(h + 1) * P]
                        for r in rg:
                            off, w = r
                            nc.tensor.matmul(psg[r][:], lhsT,
                                             at[k][:, off:off + w],
                                             start=(k == 0), stop=(k == MF - 1))
                    for r in rg:
                        off, w = r
                        if fb < NFB - 1:
                            nc.vector.tensor_add(y_acc[h][:, off:off + w],
                                                 y_acc[h][:, off:off + w],
                                                 psg[r][:])
                        else:
                            # fused finalize: y = (psum + b2) + y_acc, then
                            # scale by the gate weight and store this range
                            nc.vector.scalar_tensor_tensor(
                                y_acc[h][:, off:off + w], psg[r][:],
                                b2_sb[:, h:h + 1], y_acc[h][:, off:off + w],
                                OP.add, OP.add)
                            nc.vector.tensor_mul(y_acc[h][:, off:off + w],
                                                 y_acc[h][:, off:off + w],
                                                 comb_b[:, off:off + w])
                            nc.sync.dma_start(
                                d_yr[:, h:h + 1, off:off + w],
                                ybig[:, h:h + 1, off:off + w])

    nc.compile()
    _BUILD_CACHE[C] = nc
    return nc


def _prepare(x, Wg, alpha, ln_w, ln_b, fc1_w, fc1_b, fc2_w, fc2_b):
    """Host-side routing + per-core input construction."""
    bfnp = ml_dtypes.bfloat16
    xf = np.asarray(x, np.float32).reshape(T, H)
    Wg = np.asarray(Wg, np.float32)
    alpha = np.asarray(alpha, np.float32)
    ln_w = np.asarray(ln_w, np.float32)
    ln_b = np.asarray(ln_b, np.float32)
    fc1_w = np.asarray(fc1_w, np.float32)
    fc1_b = np.asarray(fc1_b, np.float32)
    fc2_w = np.asarray(fc2_w, np.float32)
    fc2_b = np.asarray(fc2_b, np.float32)

    logits = xf @ Wg
    order = np.argsort(-logits, axis=1, kind="stable")
    top2 = order[:, :TOP_K]
    tv = np.take_along_axis(logits, top2, 1)
    sm = np.exp(tv - tv.max(1, keepdims=True))
    sm /= sm.sum(1, keepdims=True)
    comb = np.zeros((T, E), np.float32)
    np.put_along_axis(comb, top2, sm, 1)
    comb *= alpha
    sel = np.zeros((T, E), dtype=bool)
    sel[np.arange(T)[:, None], top2] = True
    idx = [np.nonzero(sel[:, e])[0] for e in range(E)]

    maxc = max(len(i) for i in idx)
    C = max(512, 16 * ((maxc + 15) // 16))

    in_maps = []
    for e in range(E):
        n = len(idx[e])
        xg = np.zeros((C, H), np.float32)
        xg[:n] = xf[idx[e]]
        cv = np.zeros((1, C), bfnp)
        cv[0, :n] = comb[idx[e], e].astype(bfnp)
        # LayerNorm folded into fc1 (see module docstring)
        w1e = ln_w[e][:, None] * fc1_w[e]
        s1 = w1e.sum(0)
        b1p = fc1_b[e] + ln_b[e] @ fc1_w[e]
        in_maps.append({
            "xgT": np.ascontiguousarray(xg.T).astype(bfnp),
            "w1": w1e.astype(bfnp),
            "w2": fc2_w[e].astype(bfnp),
            "b1r": np.ascontiguousarray(b1p.reshape(F // 128, 128).T),
            "s1r": np.ascontiguousarray(s1.reshape(F // 128, 128).T),
            "b2": np.ascontiguousarray(fc2_b[e].reshape(KH, 128).T),
            "comb": cv,
        })
    return in_maps, idx, C


def _kernel_impl(inputs, trace=False, trace_cores=None):
    from concourse import bass_utils

    in_maps, idx, C = _prepare(**inputs)
    nc = _build(C)
    res = bass_utils.run_bass_kernel_spmd(
        nc, in_maps, core_ids=list(range(E)),
        trace=trace, trace_cores=trace_cores)

    out = np.zeros((T, H), np.float32)
    for e in range(E):
        yt = np.asarray(res.results[e]["ytT"], np.float32)  # [H, C]
        n = len(idx[e])
        out[idx[e]] += yt.T[:n]
    return out.reshape(B, S, H), res


def kernel(**inputs):
    out, _ = _kernel_impl(inputs)
    return out


# revision 30
# speedup vs baseline: 1.0181x; 1.0133x over previous
"""Trainium2 Bass kernel for a top-2 gated MoE layer (8 experts, H=1024, F=4096).

Strategy (expert parallelism across the 8 NeuronCores):
  - Host computes the top-2 routing AND the combine weights
    comb = softmax(top2) * alpha, gathers each expert's tokens into a padded
    capacity-C block, and pre-arranges EVERY device transfer as its own
    contiguous DRAM slab (x per column-chunk, weights per piece/block, all
    already in the SBUF [partition, k, col] layout).  Contiguous slabs keep
    each DMA at full HBM rate and give piece-granular arrival semaphores.
  - LayerNorm is folded into fc1 algebraically:
        hdn @ W1 = inv * (x @ W1e - mu * s1) + (lnb @ W1)
    with W1e = lnw[:,None] * W1 and s1 = colsum(W1e) precomputed on host.
    fc1 matmuls consume raw x tiles; the per-token mean/inv statistics only
    enter a cheap post-matmul fixup on the PSUM output before the gelu.
  - Each core runs one expert (matmuls in bf16, fp32 PSUM accumulation).
  - Host scatter-adds the per-expert outputs back into the full [B,S,H].

Self-contained: shapes are hardcoded from the problem spec.
"""

import numpy as np
import ml_dtypes
from contextlib import ExitStack

TOP_K = 2
LN_EPS = 1e-5
B, S, H, E, F = 2, 2048, 1024, 8, 4096
T = B * S
P = 128
KH = H // P          # 8 H-tiles
FB = 1024            # F block size
NFB = F // FB        # 4 blocks
MF = FB // P         # 8 F-tiles per block
NQ = 4               # block-0 weight quarters

_BUILD_CACHE = {}


def _chunks(C):
    # First chunk sized so fb0(c0) PE work covers the weight DMA stream;
    # the remainder chunk lands last (small finalize tail).
    out = []
    off = 0
    if C >= 960:
        out.append((0, 448))
        off = 448
    while C - off > 512:
        out.append((off, 512))
        off += 512
    if C - off:
        out.append((off, C - off))
    return out


def _build(C):
    """Build + compile the single-core Bass program (SPMD across 8 cores)."""
    if C in _BUILD_CACHE:
        return _BUILD_CACHE[C]

    import concourse.bass as bass  # noqa: F401
    import concourse.tile as tile
    import concourse.mybir as mybir
    from concourse import bacc, bass_isa  # noqa: F401

    bf = mybir.dt.bfloat16
    f32 = mybir.dt.float32
    AF = mybir.ActivationFunctionType
    OP = mybir.AluOpType

    nc = bacc.Bacc("TRN2", target_bir_lowering=False, debug=False, num_devices=8)

    chunks = _chunks(C)
    d_x = [nc.dram_tensor(f"x{ci}", [P, KH, w], bf, kind="ExternalInput")
           for ci, (off, w) in enumerate(chunks)]
    d_w1q = [nc.dram_tensor(f"w1q{q}", [P, KH, FB // NQ], bf,
                            kind="ExternalInput") for q in range(NQ)]
    d_w2q = [nc.dram_tensor(f"w2q{q}", [P, MF, H // NQ], bf,
                            kind="ExternalInput") for q in range(NQ)]
    d_w1b = [nc.dram_tensor(f"w1b{fb}", [P, KH, FB], bf,
                            kind="ExternalInput") for fb in range(1, NFB)]
    d_w2b = [nc.dram_tensor(f"w2b{fb}", [P, MF, H], bf,
                            kind="ExternalInput") for fb in range(1, NFB)]
    d_b1r = nc.dram_tensor("b1r", [P, F // P], f32, kind="ExternalInput")
    d_s1r = nc.dram_tensor("s1r", [P, F // P], f32, kind="ExternalInput")
    d_b2 = nc.dram_tensor("b2", [P, KH], f32, kind="ExternalInput")
    d_comb = nc.dram_tensor("comb", [1, C], bf, kind="ExternalInput")
    d_y = [nc.dram_tensor(f"y{ci}", [P, KH, w], bf, kind="ExternalOutput")
           for ci, (off, w) in enumerate(chunks)]

    with tile.TileContext(nc) as tc, ExitStack() as ctx:
        const = ctx.enter_context(tc.tile_pool(name="const", bufs=1))
        gpool = ctx.enter_context(tc.tile_pool(name="gate", bufs=1))
        bpool = ctx.enter_context(tc.tile_pool(name="bcast", bufs=1))
        xpool = ctx.enter_context(tc.tile_pool(name="x", bufs=1))
        w1a = ctx.enter_context(tc.tile_pool(name="w1a", bufs=NQ))
        w2a = ctx.enter_context(tc.tile_pool(name="w2a", bufs=NQ))
        sqpool = ctx.enter_context(tc.tile_pool(name="sq", bufs=8))
        tpool = ctx.enter_context(tc.tile_pool(name="t1", bufs=3))
        fxpool = ctx.enter_context(tc.tile_pool(name="fx", bufs=4))
        w1pool = ctx.enter_context(tc.tile_pool(name="w1", bufs=2))
        w2pool = ctx.enter_context(tc.tile_pool(name="w2", bufs=2))
        apool = ctx.enter_context(tc.tile_pool(name="acts", bufs=8))
        ypool = ctx.enter_context(tc.tile_pool(name="yacc", bufs=1))
        ps_small = ctx.enter_context(
            tc.tile_pool(name="ps_small", bufs=2, space="PSUM"))
        ps1 = ctx.enter_context(tc.tile_pool(name="ps1", bufs=3, space="PSUM"))
        ps2 = ctx.enter_context(tc.tile_pool(name="ps2", bufs=3, space="PSUM"))

        # ---- constants ----
        ones_k = const.tile([P, 1], bf)
        nc.vector.memset(ones_k, 1.0)
        # all-ones [128,128] lhsT: column-reduces AND partition-broadcasts
        # in one matmul (out[m,c] = sum_k x[k,c] for every m)
        ones_mat = const.tile([P, P], bf)
        nc.vector.memset(ones_mat, 1.0)
        glib0 = const.tile([E, 1], f32)
        glib1 = const.tile([E, 1], f32)
        glibs = const.tile([E, 1], f32)
        glibr = const.tile([E, 1], f32)
        nc.vector.memset(glib0, 1.0)
        # pre-warm: gpsimd partition-op library, the Sqrt ACT table, and the
        # custom-DVE reciprocal uops — all during the initial x DMA wait
        nc.gpsimd.partition_broadcast(glib1[:], glib0[0:1, :], E)
        nc.scalar.activation(glibs[:], glib0[:], AF.Sqrt)
        nc.vector.reciprocal_approx_fast(out=glibr[:], in_=glib0[:])
        # PE warm-up: junk matmuls train the HAM clock gate toward 2.4 GHz
        # while the first x/w DMAs are still in flight.
        warm_rhs = const.tile([P, 512], bf)
        nc.vector.memset(warm_rhs, 0.0)
        ps_w = ps_small.tile([1, 512], f32, tag="pss", name="warm")
        for i in range(12):
            nc.tensor.matmul(ps_w[:], ones_k[:], warm_rhs[:],
                             start=True, stop=True)

        # ---- DMA in strict first-use order on the sync ring; every
        # transfer is a whole contiguous DRAM slab ----
        xt = [xpool.tile([P, KH, w], bf, tag=f"xc{ci}", name=f"x_{ci}")
              for ci, (off, w) in enumerate(chunks)]

        def xk(ci, k):
            return xt[ci][:, k, :]

        nc.sync.dma_start(xt[0][:], d_x[0].ap())
        w1q = []
        for q in range(NQ):
            t = w1a.tile([P, KH, FB // NQ], bf, tag="w1a", name=f"w1a_{q}")
            nc.sync.dma_start(t[:], d_w1q[q].ap())
            w1q.append(t)
        w2q = []
        for q in range(2):
            t = w2a.tile([P, MF, H // NQ], bf, tag="w2a", name=f"w2a_{q}")
            nc.sync.dma_start(t[:], d_w2q[q].ap())
            w2q.append(t)
        for ci in range(1, len(chunks)):
            nc.sync.dma_start(xt[ci][:], d_x[ci].ap())
        for q in range(2, NQ):
            t = w2a.tile([P, MF, H // NQ], bf, tag="w2a", name=f"w2a_{q}")
            nc.sync.dma_start(t[:], d_w2q[q].ap())
            w2q.append(t)

        b2_sb = const.tile([P, KH], f32)
        nc.sync.dma_start(b2_sb[:], d_b2.ap())
        b1_sb = const.tile([P, F // P], f32)
        nc.sync.dma_start(b1_sb[:], d_b1r.ap())
        s1_sb = const.tile([P, F // P], f32)
        nc.sync.dma_start(s1_sb[:], d_s1r.ap())
        comb_row = gpool.tile([1, C], bf)
        nc.sync.dma_start(comb_row[:], d_comb.ap())
        comb_b = bpool.tile([P, C], bf)
        nc.gpsimd.partition_broadcast(comb_b[:], comb_row[:], P)

        eps_t = gpool.tile([P, 1], f32)
        nc.vector.memset(eps_t, float(LN_EPS))
        m_b = bpool.tile([P, C], bf)
        inv_b = bpool.tile([P, C], f32)

        ybig = ypool.tile([P, KH, C], bf, tag="yacc", name="ybig")
        y_acc = [ybig[:, h, :] for h in range(KH)]

        def load_w_block(fb):
            w1blk = w1pool.tile([P, KH, FB], bf, tag="w1", name=f"w1_{fb}")
            nc.sync.dma_start(w1blk[:], d_w1b[fb - 1].ap())
            w2blk = w2pool.tile([P, MF, H], bf, tag="w2", name=f"w2_{fb}")
            nc.sync.dma_start(w2blk[:], d_w2b[fb - 1].ap())
            return ([w1blk[:, k, :] for k in range(KH)],
                    [w2blk[:, k, :] for k in range(MF)])

        at0 = [apool.tile([P, C], bf, tag="acts", name=f"a_0_{m}")
               for m in range(MF)]

        def emit_prologue(ci):
            off, w = chunks[ci]
            # x^2 split between DVE and ACT so neither engine gates the chain
            sqs = []
            for k in range(KH):
                sq_c = sqpool.tile([P, w], bf, tag="sq", name=f"sq_{off}_{k}")
                if k < 4:
                    nc.vector.tensor_mul(sq_c[:], xk(ci, k), xk(ci, k))
                else:
                    nc.scalar.activation(sq_c[:], xk(ci, k), AF.Square)
                sqs.append(sq_c)
            # fold the 8 H-tiles on the DVE (bf16 partials), then a single
            # all-ones matmul per stat reduces across partitions AND
            # broadcasts the result to every partition
            px = tpool.tile([P, w], bf, tag="px", name=f"px_{off}")
            psq = tpool.tile([P, w], bf, tag="psq", name=f"psq_{off}")
            nc.vector.tensor_add(px[:], xk(ci, 0), xk(ci, 1))
            for k in range(2, KH):
                nc.vector.tensor_add(px[:], px[:], xk(ci, k))
            nc.vector.tensor_add(psq[:], sqs[0][:], sqs[1][:])
            for k in range(2, KH):
                nc.vector.tensor_add(psq[:], psq[:], sqs[k][:])
            ps_a = ps_small.tile([P, w], f32, tag="pss", name=f"ps_sl{off}")
            nc.tensor.matmul(ps_a[:], ones_mat[:], px[:], start=True, stop=True)
            nc.scalar.activation(m_b[:, off:off + w], ps_a[:], AF.Identity,
                                 bias=0.0, scale=1.0 / H)
            ps_b = ps_small.tile([P, w], f32, tag="pss", name=f"ps_sq{off}")
            nc.tensor.matmul(ps_b[:], ones_mat[:], psq[:], start=True, stop=True)

            # var = sumsq/H - mean^2 ; inv = 1/sqrt(var + eps)
            v0 = tpool.tile([P, w], f32, tag="t1", name=f"v0_{off}")
            nc.vector.scalar_tensor_tensor(v0[:], m_b[:, off:off + w], -1.0,
                                           m_b[:, off:off + w],
                                           OP.mult, OP.mult)
            nc.vector.scalar_tensor_tensor(v0[:], ps_b[:], 1.0 / H,
                                           v0[:], OP.mult, OP.add)
            nc.scalar.activation(v0[:], v0[:], AF.Sqrt,
                                 bias=eps_t[:], scale=1.0)
            nc.vector.reciprocal_approx_fast(out=inv_b[:, off:off + w],
                                             in_=v0[:])

        def fc1_fixup(psum, dst, off, w, fcol):
            # gelu( inv*(A - mu*s1) + b1' ):
            #   t = (m_b * s1_f) - A ; t = t * inv ; ACT gelu(-t + b1')
            sl = slice(off, off + w)
            tmp = fxpool.tile([P, w], f32, tag="fx", name=f"fx_{fcol}_{off}")
            nc.vector.scalar_tensor_tensor(tmp[:], m_b[:, sl],
                                           s1_sb[:, fcol:fcol + 1],
                                           psum[:], OP.mult, OP.subtract)
            nc.vector.tensor_mul(tmp[:], tmp[:], inv_b[:, sl])
            nc.scalar.activation(dst, tmp[:], AF.Gelu_apprx_tanh,
                                 bias=b1_sb[:, fcol:fcol + 1], scale=-1.0)

        def emit_fb0(ci):
            # F-block 0 fc1 -> fixup+gelu -> fc2 on this chunk, quarter-split
            # weights so compute starts as soon as each piece lands
            off, w = chunks[ci]
            sl = slice(off, off + w)
            for m in range(MF):
                w1piece = w1q[m // 2]
                msl = slice((m % 2) * P, (m % 2) * P + P)
                pst = ps1.tile([P, w], f32, tag="ps1", name=f"ps1_0_{m}_{ci}")
                for k in range(KH):
                    nc.tensor.matmul(pst[:], w1piece[:, k, msl],
                                     xk(ci, k),
                                     start=(k == 0), stop=(k == KH - 1))
                fc1_fixup(pst, at0[m][:, sl], off, w, m)
            for h in range(KH):
                w2piece = w2q[h // 2]
                hsl = slice((h % 2) * P, (h % 2) * P + P)
                pst = ps2.tile([P, w], f32, tag="ps2", name=f"ps2_0_{h}_{ci}")
                for k in range(MF):
                    nc.tensor.matmul(pst[:], w2piece[:, k, hsl],
                                     at0[k][:, sl],
                                     start=(k == 0), stop=(k == MF - 1))
                nc.scalar.activation(y_acc[h][:, sl], pst[:], AF.Identity,
                                     bias=0.0)

        # natural order: the x-chunk DMA gates each prologue anyway, and the
        # stats only feed the post-matmul fixups, never the matmuls
        for ci in range(len(chunks)):
            emit_prologue(ci)
            emit_fb0(ci)

        # ---- Phase D: remaining F blocks.  Middle blocks iterate
        # weight-stationary (each lhsT feeds all chunks); the last block
        # iterates per-range so the finalize tail is short. ----
        for fb in range(1, NFB):
            w1t, w2t = load_w_block(fb)

            at = [apool.tile([P, C], bf, tag="acts", name=f"a_{fb}_{m}")
                  for m in range(MF)]
            if fb == NFB - 1:
                order = sorted(range(len(chunks)),
                               key=lambda ci: -chunks[ci][1])
                groups = [[ci] for ci in order]
            else:
                groups = [list(range(len(chunks)))]

            for cig in groups:
                for m in range(MF):
                    psg = {ci: ps1.tile([P, chunks[ci][1]], f32, tag="ps1",
                                        name=f"ps1_{fb}_{m}_{ci}")
                           for ci in cig}
                    for k in range(KH):
                        lhsT = w1t[k][:, m * P:(m + 1) * P]
                        for ci in cig:
                            nc.tensor.matmul(psg[ci][:], lhsT, xk(ci, k),
                                             start=(k == 0), stop=(k == KH - 1))
                    fcol = fb * MF + m
                    for ci in cig:
                        off, w = chunks[ci]
                        fc1_fixup(psg[ci], at[m][:, off:off + w], off, w, fcol)
                for h in range(KH):
                    psg = {ci: ps2.tile([P, chunks[ci][1]], f32, tag="ps2",
                                        name=f"ps2_{fb}_{h}_{ci}")
                           for ci in cig}
                    for k in range(MF):
                        lhsT = w2t[k][:, h * P:(h + 1) * P]
                        for ci in cig:
                            off, w = chunks[ci]
                            nc.tensor.matmul(psg[ci][:], lhsT,
                                             at[k][:, off:off + w],
                                             start=(k == 0), stop=(k == MF - 1))
                    for ci in cig:
                        off, w = chunks[ci]
                        if fb < NFB - 1:
                            nc.vector.tensor_add(y_acc[h][:, off:off + w],
                                                 y_acc[h][:, off:off + w],
                                                 psg[ci][:])
                        else:
                            # fused finalize: y = (psum + b2) + y_acc, then
                            # scale by the gate weight
                            nc.vector.scalar_tensor_tensor(
                                y_acc[h][:, off:off + w], psg[ci][:],
                                b2_sb[:, h:h + 1], y_acc[h][:, off:off + w],
                                OP.add, OP.add)
                            nc.vector.tensor_mul(y_acc[h][:, off:off + w],
                                                 y_acc[h][:, off:off + w],
                                                 comb_b[:, off:off + w])
                if fb == NFB - 1:
                    # one contiguous slab store per range
                    ci = cig[0]
                    off, w = chunks[ci]
                    nc.sync.dma_start(d_y[ci].ap(),
                                      ybig[:, :, off:off + w])

    nc.compile()
    _BUILD_CACHE[C] = nc
    return nc


def _prepare(x, Wg, alpha, ln_w, ln_b, fc1_w, fc1_b, fc2_w, fc2_b):
    """Host-side routing + per-core slab construction."""
    bfnp = ml_dtypes.bfloat16
    xf = np.asarray(x, np.float32).reshape(T, H)
    Wg = np.asarray(Wg, np.float32)
    alpha = np.asarray(alpha, np.float32)
    ln_w = np.asarray(ln_w, np.float32)
    ln_b = np.asarray(ln_b, np.float32)
    fc1_w = np.asarray(fc1_w, np.float32)
    fc1_b = np.asarray(fc1_b, np.float32)
    fc2_w = np.asarray(fc2_w, np.float32)
    fc2_b = np.asarray(fc2_b, np.float32)

    logits = xf @ Wg
    order = np.argsort(-logits, axis=1, kind="stable")
    top2 = order[:, :TOP_K]
    tv = np.take_along_axis(logits, top2, 1)
    sm = np.exp(tv - tv.max(1, keepdims=True))
    sm /= sm.sum(1, keepdims=True)
    comb = np.zeros((T, E), np.float32)
    np.put_along_axis(comb, top2, sm, 1)
    comb *= alpha
    sel = np.zeros((T, E), dtype=bool)
    sel[np.arange(T)[:, None], top2] = True
    idx = [np.nonzero(sel[:, e])[0] for e in range(E)]

    maxc = max(len(i) for i in idx)
    C = max(512, 16 * ((maxc + 15) // 16))
    chunks = _chunks(C)

    in_maps = []
    for e in range(E):
        n = len(idx[e])
        xg = np.zeros((C, H), bfnp)
        xg[:n] = xf[idx[e]].astype(bfnp)
        # [KH, P, C] view of x^T, then per-chunk [P, KH, w] slabs
        xr = np.ascontiguousarray(xg.T).reshape(KH, P, C)
        cv = np.zeros((1, C), bfnp)
        cv[0, :n] = comb[idx[e], e].astype(bfnp)
        # LayerNorm folded into fc1 (see module docstring)
        w1e = ln_w[e][:, None] * fc1_w[e]
        s1 = w1e.sum(0)
        b1p = fc1_b[e] + ln_b[e] @ fc1_w[e]
        w1r = w1e.astype(bfnp).reshape(KH, P, F)
        w2r = fc2_w[e].astype(bfnp).reshape(F // P, P, H)
        im = {
            "b1r": np.ascontiguousarray(b1p.reshape(F // P, P).T),
            "s1r": np.ascontiguousarray(s1.reshape(F // P, P).T),
            "b2": np.ascontiguousarray(fc2_b[e].reshape(KH, P).T),
            "comb": cv,
        }
        for ci, (off, w) in enumerate(chunks):
            im[f"x{ci}"] = np.ascontiguousarray(
                xr[:, :, off:off + w].transpose(1, 0, 2))
        WQ = FB // NQ
        for q in range(NQ):
            im[f"w1q{q}"] = np.ascontiguousarray(
                w1r[:, :, q * WQ:(q + 1) * WQ].transpose(1, 0, 2))
        HQ = H // NQ
        for q in range(NQ):
            im[f"w2q{q}"] = np.ascontiguousarray(
                w2r[0:MF, :, q * HQ:(q + 1) * HQ].transpose(1, 0, 2))
        for fb in range(1, NFB):
            im[f"w1b{fb}"] = np.ascontiguousarray(
                w1r[:, :, fb * FB:(fb + 1) * FB].transpose(1, 0, 2))
            im[f"w2b{fb}"] = np.ascontiguousarray(
                w2r[fb * MF:(fb + 1) * MF].transpose(1, 0, 2))
        in_maps.append(im)
    return in_maps, idx, C


def _kernel_impl(inputs, trace=False, trace_cores=None):
    from concourse import bass_utils

    in_maps, idx, C = _prepare(**inputs)
    chunks = _chunks(C)
    nc = _build(C)
    res = bass_utils.run_bass_kernel_spmd(
        nc, in_maps, core_ids=list(range(E)),
        trace=trace, trace_cores=trace_cores)

    out = np.zeros((T, H), np.float32)
    for e in range(E):
        yt = np.empty((H, C), np.float32)  # [H, C]
        for ci, (off, w) in enumerate(chunks):
            slab = np.asarray(res.results[e][f"y{ci}"], np.float32)
            yt[:, off:off + w] = slab.transpose(1, 0, 2).reshape(H, w)
        n = len(idx[e])
        out[idx[e]] += yt.T[:n]
    return out.reshape(B, S, H), res


def kernel(**inputs):
    out, _ = _kernel_impl(inputs)
    return out
